# revision 19
# baseline (speedup 1.0000x reference)
"""Distributed kNN-retrieval kernel for Trainium2 (8 NeuronCores).

Problem: nn_CHRC_47562467836574 (retrieval_knn).
  corrected[b] = softmax-weighted sum of values rows at the top-16
  decayed cosine similarities between query b and a 100k-entry memory bank.

Strategy (8-way SPMD, bass/Tile):
  * Decay cutoff: timestamps are sorted, and with L2-normalized vectors
    |cos| <= 1, so an entry's decayed sim is bounded by its decay factor
    0.995^age.  Entries older than the cutoff (decay < CUT) can never reach
    a query's top-16 (16th-best sims measure ~0.08 on this distribution).
    The host keeps only the newest slice (rounded up to a full multiple of
    8*TILE_N), trimming ~80% of the matmul work.  A post-hoc host check
    verifies 16th-best >= CUT for every query (exact CPU recompute of any
    violating row — never triggers in practice).
  * Keys: newest slice, sharded contiguously across 8 cores; each core
    computes exact fp32 sims for all 1024 queries x its shard, with
    normalization + decay folded into a per-column key prescale.
  * Local top-16 per query via per-tile vector-engine max8/max_index
    (top-16 per 500-wide tile can never miss a local-top-16 member).
  * AllGather of the 8x(local top-16 sims + global indices); every core
    reduces to the global top-16, softmax-weights them, and gathers the
    value rows for its OWN 12-head slice of values (values are sharded by
    head).  Output slices are concatenated on host.
"""

import math
import os

import numpy as np

CUT = 0.01          # decay cutoff; 16th-best sims ~0.08 on this data (7x margin)
DECAY_FACTOR = 0.995
TEMPERATURE = 0.1
MIN_SIMILARITY = 0.0
EPS = 1e-8

_cache = {}


# ---------------------------------------------------------------------------
# device program
# ---------------------------------------------------------------------------

def build(b, n_loc, n_rows, hf_c, tile_n=500, n_cores=8, d=512, k=16):
    """Build + compile the SPMD program (same program for every core)."""
    from contextlib import ExitStack

    import concourse.bass as bass
    import concourse.tile as tile
    from concourse import bacc, mybir

    f32 = mybir.dt.float32
    u32 = mybir.dt.uint32
    nt = n_loc // tile_n
    assert n_loc % tile_n == 0
    nb = b // 128
    assert b % 128 == 0
    nt16 = nt * 16
    ln_decay = float(np.log(np.float32(DECAY_FACTOR)))

    nc = bacc.Bacc("TRN2", target_bir_lowering=False, debug=False,
                   num_devices=n_cores)

    qT = nc.dram_tensor("qT", [d, b], f32, kind="ExternalInput")
    kT = nc.dram_tensor("kT", [d, n_loc], f32, kind="ExternalInput")
    age = nc.dram_tensor("age", [1, n_loc], f32, kind="ExternalInput")
    vals = nc.dram_tensor("vals", [n_rows, hf_c], f32, kind="ExternalInput")
    crow = nc.dram_tensor("crow", [128, 1], u32, kind="ExternalInput")
    iota_t = nc.dram_tensor("iota_t", [1, nt16], f32, kind="ExternalInput")
    iota_g = nc.dram_tensor("iota_g", [1, n_cores * k], f32, kind="ExternalInput")
    out = nc.dram_tensor("out", [b, hf_c], f32, kind="ExternalOutput")
    dbg_lv = nc.dram_tensor("dbg_lv", [b, k], f32, kind="ExternalOutput")
    dbg_gx = nc.dram_tensor("dbg_gx", [b, k], u32, kind="ExternalOutput")
    dbg_ags = nc.dram_tensor("dbg_ags", [n_cores * b, k], f32, kind="ExternalOutput")
    dbg_agi = nc.dram_tensor("dbg_agi", [n_cores * b, k], u32, kind="ExternalOutput")
    dbg_s = nc.dram_tensor("dbg_s", [b, k], f32, kind="ExternalOutput")
    dbg_i = nc.dram_tensor("dbg_i", [b, k], u32, kind="ExternalOutput")

    dch = d // 128  # 4 contraction chunks

    with tile.TileContext(nc) as tc, ExitStack() as ctx:
        sb = ctx.enter_context(tc.tile_pool(name="sb", bufs=1))
        sb3 = ctx.enter_context(tc.tile_pool(name="sb3", bufs=3))
        sb2 = ctx.enter_context(tc.tile_pool(name="sb2", bufs=2))
        ps = ctx.enter_context(tc.tile_pool(name="ps", bufs=3, space="PSUM"))
        psn = ctx.enter_context(tc.tile_pool(name="psn", bufs=2, space="PSUM"))
        dram = ctx.enter_context(tc.tile_pool(name="dram", bufs=1, space="DRAM"))

        # ---- constants / loads -------------------------------------------
        ones = sb.tile([128, 128], f32, tag="ones")
        nc.vector.memset(ones[:], 1.0)
        crow_s = sb.tile([128, 1], u32, tag="crow")
        nc.sync.dma_start(out=crow_s[:], in_=crow.ap())
        # replicated iota rows for one-hot free-axis gathers
        iota_t_s = sb.tile([128, nt16], f32, tag="iota_t")
        nc.sync.dma_start(out=iota_t_s[:], in_=iota_t.ap().to_broadcast([128, nt16]))
        iota_g_s = sb.tile([128, n_cores * k], f32, tag="iota_g")
        nc.sync.dma_start(out=iota_g_s[:],
                          in_=iota_g.ap().to_broadcast([128, n_cores * k]))
        qTs = sb.tile([128, dch, b], f32, tag="qT")
        nc.sync.dma_start(out=qTs[:], in_=qT.ap().rearrange("(c p) b -> p c b", p=128))
        kts = []
        for t in range(nt):
            kt_t = sb.tile([128, dch, tile_n], f32, tag=f"kt{t}")
            nc.sync.dma_start(
                out=kt_t[:],
                in_=kT.ap().rearrange("(c p) n -> p c n", p=128)[
                    :, :, t * tile_n:(t + 1) * tile_n],
            )
            kts.append(kt_t)

        # ---- query normalization -----------------------------------------
        # ones[128,128] stationary => norm sums replicated on all partitions
        qnrm = sb.tile([128, b], f32, tag="qnrm")
        nbt = math.ceil(b / 512)
        for i in range(nbt):
            w = min(512, b - i * 512)
            sq_q = sb2.tile([128, dch, w], f32, tag="sqx", name="sq_q")
            nc.scalar.square(sq_q[:], qTs[:, :, i * 512:i * 512 + w])
            pq = psn.tile([128, w], f32, tag="pn")
            for c in range(dch):
                nc.tensor.matmul(pq[:], ones[:], sq_q[:, c, :],
                                 start=(c == 0), stop=(c == dch - 1))
            nc.scalar.sqrt(qnrm[:, i * 512:i * 512 + w], pq[:])
        nc.vector.tensor_scalar_max(qnrm[:], qnrm[:], 1e-12)
        qinv = sb.tile([128, b], f32, tag="qinv")
        nc.vector.reciprocal(qinv[:], qnrm[:])
        nc.vector.tensor_tensor(
            out=qTs[:], in0=qTs[:],
            in1=qinv[:].unsqueeze(1).to_broadcast([128, dch, b]),
            op=mybir.AluOpType.mult)

        # ---- per-tile: key prescale (norm * decay), sims, local scan -----
        vbufs, pbufs = [], []
        for bc in range(nb):
            vbufs.append(sb.tile([128, nt16], f32, tag=f"vb{bc}", name=f"vb{bc}"))
            pbufs.append(sb.tile([128, nt16], u32, tag=f"pb{bc}", name=f"pb{bc}"))

        for t in range(nt):
            kt_t = kts[t]
            sq_k = sb2.tile([128, dch, tile_n], f32, tag="sqx", name="sq_k")
            nc.scalar.square(sq_k[:], kt_t[:])
            pn = psn.tile([128, tile_n], f32, tag="pn")
            for c in range(dch):
                nc.tensor.matmul(pn[:], ones[:], sq_k[:, c, :],
                                 start=(c == 0), stop=(c == dch - 1))
            knrm = sb2.tile([128, tile_n], f32, tag="knrm")
            nc.scalar.sqrt(knrm[:], pn[:])
            nc.vector.tensor_scalar_max(knrm[:], knrm[:], 1e-12)
            kinv = sb2.tile([128, tile_n], f32, tag="kinv")
            nc.vector.reciprocal(kinv[:], knrm[:])
            aget = sb2.tile([128, tile_n], f32, tag="aget")
            nc.sync.dma_start(
                out=aget[:],
                in_=age.ap()[:, t * tile_n:(t + 1) * tile_n]
                    .to_broadcast([128, tile_n]))
            dec = sb2.tile([128, tile_n], f32, tag="dec")
            nc.scalar.activation(dec[:], aget[:],
                                 mybir.ActivationFunctionType.Exp,
                                 bias=0.0, scale=ln_decay)
            nc.vector.tensor_tensor(out=kinv[:], in0=kinv[:], in1=dec[:],
                                    op=mybir.AluOpType.mult)
            nc.vector.tensor_tensor(
                out=kt_t[:], in0=kt_t[:],
                in1=kinv[:].unsqueeze(1).to_broadcast([128, dch, tile_n]),
                op=mybir.AluOpType.mult)

            for bc in range(nb):
                pt = ps.tile([128, tile_n], f32, tag="p")
                for c in range(dch):
                    nc.tensor.matmul(pt[:], qTs[:, c, bc * 128:(bc + 1) * 128],
                                     kt_t[:, c, :],
                                     start=(c == 0), stop=(c == dch - 1))
                sims = sb3.tile([128, tile_n], f32, tag="sims")
                nc.scalar.copy(sims[:], pt[:])
                vb, pb = vbufs[bc], pbufs[bc]
                nc.vector.max(vb[:, t * 16:t * 16 + 8], sims[:])
                nc.vector.max_index(pb[:, t * 16:t * 16 + 8],
                                    vb[:, t * 16:t * 16 + 8], sims[:])
                scr = sb3.tile([128, tile_n], f32, tag="scr")
                nc.vector.match_replace(scr[:], vb[:, t * 16:t * 16 + 8],
                                        sims[:], -3.0e38)
                nc.vector.max(vb[:, t * 16 + 8:t * 16 + 16], scr[:])
                nc.vector.max_index(pb[:, t * 16 + 8:t * 16 + 16],
                                    vb[:, t * 16 + 8:t * 16 + 16], scr[:])

        # ---- local top-16 + global indices -------------------------------
        ag_in_s = dram.tile([b, k], f32, tag="ag_in_s")
        ag_in_i = dram.tile([b, k], u32, tag="ag_in_i")

        for bc in range(nb):
            vb, pb = vbufs[bc], pbufs[bc]
            lv = sb3.tile([128, k], f32, tag="lv")
            nc.vector.max(lv[:, 0:8], vb[:])
            vscr = sb2.tile([128, nt16], f32, tag="vscr")
            nc.vector.match_replace(vscr[:], lv[:, 0:8], vb[:], -3.0e38)
            nc.vector.max(lv[:, 8:16], vscr[:])
            vp = sb3.tile([128, k], u32, tag="vp")
            nc.vector.max_index(vp[:, 0:8], lv[:, 0:8], vb[:])
            nc.vector.max_index(vp[:, 8:16], lv[:, 8:16], vscr[:])
            # one-hot gather lpos[p,j] = pb[p, vp[p,j]]  (per-partition idx;
            # fp32 throughout: values < 2^24 so integer-exact)
            vp_f = sb3.tile([128, k], f32, tag="vp_f")
            nc.vector.tensor_copy(out=vp_f[:], in_=vp[:])
            pb_f = sb3.tile([128, nt16], f32, tag="pb_f")
            nc.vector.tensor_copy(out=pb_f[:], in_=pb[:])
            cmp = sb2.tile([128, k, nt16], f32, tag="cmp")
            nc.vector.tensor_tensor(
                out=cmp[:],
                in0=vp_f[:].unsqueeze(2).to_broadcast([128, k, nt16]),
                in1=iota_t_s[:].unsqueeze(1).to_broadcast([128, k, nt16]),
                op=mybir.AluOpType.is_equal)
            nc.vector.tensor_tensor(
                out=cmp[:], in0=cmp[:],
                in1=pb_f[:].unsqueeze(1).to_broadcast([128, k, nt16]),
                op=mybir.AluOpType.mult)
            lpos_f = sb3.tile([128, k], f32, tag="lpos_f")
            nc.vector.tensor_reduce(lpos_f[:], cmp[:],
                                    axis=mybir.AxisListType.X,
                                    op=mybir.AluOpType.add)
            lpos = sb3.tile([128, k], u32, tag="lpos")
            nc.vector.tensor_copy(out=lpos[:], in_=lpos_f[:])
            # gidx = (vp>>4)*tile_n + lpos + crow
            gidx = sb3.tile([128, k], u32, tag="gidx")
            nc.vector.tensor_scalar(out=gidx[:], in0=vp[:], scalar1=4,
                                    scalar2=None,
                                    op0=mybir.AluOpType.logical_shift_right)
            nc.vector.tensor_scalar(out=gidx[:], in0=gidx[:], scalar1=tile_n,
                                    scalar2=None, op0=mybir.AluOpType.mult)
            nc.vector.tensor_tensor(out=gidx[:], in0=gidx[:], in1=lpos[:],
                                    op=mybir.AluOpType.add)
            nc.vector.tensor_tensor(out=gidx[:], in0=gidx[:],
                                    in1=crow_s[:].to_broadcast([128, k]),
                                    op=mybir.AluOpType.add)
            # clamp (padding-path safety; real candidates are always in range)
            nc.vector.tensor_scalar_min(gidx[:], gidx[:], n_rows - 1)
            nc.sync.dma_start(out=ag_in_s[bc * 128:(bc + 1) * 128, :], in_=lv[:])
            nc.sync.dma_start(out=ag_in_i[bc * 128:(bc + 1) * 128, :], in_=gidx[:])

        # ---- AllGather ----------------------------------------------------
        ag_out_s = dram.tile([n_cores * b, k], f32, tag="ag_out_s")
        ag_out_i = dram.tile([n_cores * b, k], u32, tag="ag_out_i")
        rg = [list(range(n_cores))]
        nc.gpsimd.collective_compute("AllGather", mybir.AluOpType.bypass,
                                     replica_groups=rg,
                                     ins=[ag_in_s[:].opt()],
                                     outs=[ag_out_s[:].opt()])
        nc.gpsimd.collective_compute("AllGather", mybir.AluOpType.bypass,
                                     replica_groups=rg,
                                     ins=[ag_in_i[:].opt()],
                                     outs=[ag_out_i[:].opt()])

        nc.sync.dma_start(out=dbg_lv.ap(), in_=ag_in_s[:])
        nc.sync.dma_start(out=dbg_gx.ap(), in_=ag_in_i[:])
        nc.sync.dma_start(out=dbg_ags.ap(), in_=ag_out_s[:])
        nc.sync.dma_start(out=dbg_agi.ap(), in_=ag_out_i[:])

        # ---- final reduction (every core: all queries, own head slice) ---
        for bc in range(nb):
            G = sb3.tile([128, n_cores, k], f32, tag="G")
            nc.sync.dma_start(
                out=G[:],
                in_=ag_out_s[:].rearrange("(r q) k -> q r k", r=n_cores)[
                    bc * 128:(bc + 1) * 128])
            Gv = G[:].rearrange("p r k -> p (r k)")
            nck = n_cores * k
            fv = sb3.tile([128, k], f32, tag="fv")
            nc.vector.max(fv[:, 0:8], Gv)
            Gscr = sb2.tile([128, nck], f32, tag="Gscr")
            nc.vector.match_replace(Gscr[:], fv[:, 0:8], Gv, -3.0e38)
            nc.vector.max(fv[:, 8:16], Gscr[:])
            fp = sb3.tile([128, k], u32, tag="fp")
            nc.vector.max_index(fp[:, 0:8], fv[:, 0:8], Gv)
            nc.vector.max_index(fp[:, 8:16], fv[:, 8:16], Gscr[:])
            # candidate global indices, same strided layout as G
            Gi = sb3.tile([128, n_cores, k], u32, tag="Gi")
            nc.sync.dma_start(
                out=Gi[:],
                in_=ag_out_i[:].rearrange("(r q) k -> q r k", r=n_cores)[
                    bc * 128:(bc + 1) * 128])
            # one-hot gather fgi[p,j] = Gi[p, fp[p,j]] (fp32, integer-exact)
            fp_f = sb3.tile([128, k], f32, tag="fp_f")
            nc.vector.tensor_copy(out=fp_f[:], in_=fp[:])
            Gi_f = sb3.tile([128, n_cores * k], f32, tag="Gi_f")
            nc.vector.tensor_copy(out=Gi_f[:], in_=Gi[:].rearrange("p r k -> p (r k)"))
            cmpf = sb2.tile([128, k, n_cores * k], f32, tag="cmpf")
            nc.vector.tensor_tensor(
                out=cmpf[:],
                in0=fp_f[:].unsqueeze(2).to_broadcast([128, k, n_cores * k]),
                in1=iota_g_s[:].unsqueeze(1).to_broadcast([128, k, n_cores * k]),
                op=mybir.AluOpType.is_equal)
            nc.vector.tensor_tensor(
                out=cmpf[:], in0=cmpf[:],
                in1=Gi_f[:].unsqueeze(1).to_broadcast([128, k, n_cores * k]),
                op=mybir.AluOpType.mult)
            fgi_f = sb3.tile([128, k], f32, tag="fgi_f")
            nc.vector.tensor_reduce(fgi_f[:], cmpf[:],
                                    axis=mybir.AxisListType.X,
                                    op=mybir.AluOpType.add)
            fgi = sb3.tile([128, k], u32, tag="fgi")
            nc.vector.tensor_copy(out=fgi[:], in_=fgi_f[:])
            # gather value rows for own head slice (one row per partition per j)
            V = sb2.tile([128, k, hf_c], f32, tag="V")
            for j in range(k):
                nc.gpsimd.indirect_dma_start(
                    out=V[:, j, :], out_offset=None,
                    in_=vals.ap(),
                    in_offset=bass.IndirectOffsetOnAxis(ap=fgi[:, j:j + 1], axis=0))
            # softmax weights with MIN_SIMILARITY mask + renorm (ref formula)
            negm = sb3.tile([128, 1], f32, tag="negm")
            nc.vector.tensor_scalar_mul(negm[:], fv[:, 0:1], -1.0 / TEMPERATURE)
            e = sb3.tile([128, k], f32, tag="e")
            nc.scalar.activation(e[:], fv[:], mybir.ActivationFunctionType.Exp,
                                 bias=negm[:], scale=1.0 / TEMPERATURE)
            m = sb3.tile([128, k], f32, tag="m")
            nc.vector.tensor_scalar(out=m[:], in0=fv[:],
                                    scalar1=MIN_SIMILARITY, scalar2=None,
                                    op0=mybir.AluOpType.is_ge)
            em = sb3.tile([128, k], f32, tag="em")
            nc.vector.tensor_tensor(out=em[:], in0=e[:], in1=m[:],
                                    op=mybir.AluOpType.mult)
            S = sb3.tile([128, 1], f32, tag="S")
            nc.vector.tensor_reduce(S[:], e[:], axis=mybir.AxisListType.X,
                                    op=mybir.AluOpType.add)
            Sm = sb3.tile([128, 1], f32, tag="Sm")
            nc.vector.tensor_reduce(Sm[:], em[:], axis=mybir.AxisListType.X,
                                    op=mybir.AluOpType.add)
            den = sb3.tile([128, 1], f32, tag="den")
            nc.vector.tensor_scalar(out=den[:], in0=S[:], scalar1=EPS,
                                    scalar2=Sm[:], op0=mybir.AluOpType.mult,
                                    op1=mybir.AluOpType.add)
            winv = sb3.tile([128, 1], f32, tag="winv")
            nc.vector.reciprocal(winv[:], den[:])
            w = sb3.tile([128, k], f32, tag="w")
            nc.vector.tensor_scalar(out=w[:], in0=em[:], scalar1=winv[:],
                                    scalar2=None, op0=mybir.AluOpType.mult)
            prod = sb2.tile([128, k, hf_c], f32, tag="prod")
            nc.vector.tensor_tensor(
                out=prod[:], in0=V[:],
                in1=w[:].unsqueeze(2).to_broadcast([128, k, hf_c]),
                op=mybir.AluOpType.mult)
            acc = sb3.tile([128, hf_c], f32, tag="acc")
            nc.vector.tensor_reduce(
                acc[:], prod[:].transpose([0, 2, 1]),
                axis=mybir.AxisListType.X, op=mybir.AluOpType.add)
            nc.sync.dma_start(out=out.ap()[bc * 128:(bc + 1) * 128, :], in_=acc[:])
            nc.sync.dma_start(out=dbg_s.ap()[bc * 128:(bc + 1) * 128, :], in_=fv[:])
            nc.sync.dma_start(out=dbg_i.ap()[bc * 128:(bc + 1) * 128, :], in_=fgi[:])

    nc.compile()
    return nc


# ---------------------------------------------------------------------------
# host wrapper
# ---------------------------------------------------------------------------

def _host_row_reference(qrow, keys, values, decay, top_k):
    """Exact CPU recompute of one query row (fallback safety net)."""
    qn = qrow / max(np.linalg.norm(qrow), 1e-12)
    kn = keys / np.maximum(
        np.linalg.norm(keys, axis=1, keepdims=True), 1e-12)
    sims = (kn @ qn).astype(np.float32) * decay
    idx = np.argpartition(-sims, top_k)[:top_k]
    idx = idx[np.argsort(-sims[idx], kind="stable")]
    ts_ = sims[idx]
    e = np.exp((ts_ - ts_.max()) / np.float32(TEMPERATURE))
    sm = e / e.sum()
    msk = ts_ >= MIN_SIMILARITY
    wgt = sm * msk
    wgt = wgt / (wgt.sum() + EPS)
    return np.einsum("k,khf->hf", wgt, values[idx]).astype(np.float32)


def kernel(query, keys, values, timestamps, global_step, top_k):
    from concourse import bass_utils

    query = np.asarray(query, dtype=np.float32)
    keys = np.asarray(keys, dtype=np.float32)
    values = np.asarray(values, dtype=np.float32)
    timestamps = np.asarray(timestamps)
    gs = int(global_step)
    top_k = int(top_k)
    assert top_k == 16, f"kernel compiled for top_k=16, got {top_k}"

    B, D = query.shape
    N = keys.shape[0]
    H, F = values.shape[1], values.shape[2]
    n_cores = 8
    tile_n = 500
    assert B % 128 == 0 and D == 512 and H % n_cores == 0
    hpc = H // n_cores
    hf_c = hpc * F

    # ---- decay cutoff (sorted timestamps) ----------------------------------
    age_cut = int(math.floor(math.log(CUT) / math.log(DECAY_FACTOR)))
    idx0 = int(np.searchsorted(timestamps, gs - age_cut, side="left"))
    keep = N - idx0
    n_loc = max(tile_n, math.ceil(keep / (n_cores * tile_n)) * tile_n)
    S = N - n_cores * n_loc
    if S < 0:
        # bank smaller than 8 tiles: pad by re-covering from index 0
        S = 0
        n_loc = math.ceil(N / (n_cores * tile_n)) * tile_n
    npad = S + n_cores * n_loc - N  # columns past the end (only if S==0 path)

    key = (B, n_loc, N, hf_c)
    if key not in _cache:
        _cache[key] = build(B, n_loc, N, hf_c, tile_n=tile_n, n_cores=n_cores)
    nc = _cache[key]

    # ---- host-side input prep ---------------------------------------------
    qT = np.ascontiguousarray(query.T)
    ages = (gs - timestamps).astype(np.float32)
    vals2d = values.reshape(N, H * F)

    in_maps = []
    for c in range(n_cores):
        lo = S + c * n_loc
        hi = lo + n_loc
        if hi <= N:
            ksl = keys[lo:hi]
            asl = ages[lo:hi]
        else:  # padding path (never hit at full scale)
            ksl = np.concatenate([keys[lo:N], np.ones((hi - N, D), np.float32)])
            asl = np.concatenate([ages[lo:N], np.full(hi - N, 1e9, np.float32)])
        in_maps.append({
            "qT": qT,
            "kT": np.ascontiguousarray(ksl.T),
            "age": np.ascontiguousarray(asl[None, :]),
            "vals": np.ascontiguousarray(
                values[:, c * hpc:(c + 1) * hpc, :].reshape(N, hf_c)),
            "crow": np.full((128, 1), lo, np.uint32),
            "iota_t": np.arange(n_loc // tile_n * 16, dtype=np.float32)[None, :],
            "iota_g": np.arange(n_cores * 16, dtype=np.float32)[None, :],
        })

    trace = os.environ.get("KNN_TRACE", "") == "1"
    res = bass_utils.run_bass_kernel_spmd(
        nc, in_maps, core_ids=list(range(n_cores)), trace=trace)
    kernel.last_exec_time_ns = res.exec_time_ns

    out = np.concatenate(
        [res.results[c]["out"].reshape(B, hpc, F) for c in range(n_cores)],
        axis=1)

    # ---- host safety net ---------------------------------------------------
    fv = res.results[0]["dbg_s"]     # [B, 16] final top sims (desc)
    fgi = res.results[0]["dbg_i"]    # [B, 16] final indices
    decay_full = np.power(np.float32(DECAY_FACTOR), ages).astype(np.float32)
    bad = (fv[:, top_k - 1] < CUT)
    # duplicate indices within a row (bit-equal sims tie pathology)
    srt = np.sort(fgi, axis=1)
    bad |= (srt[:, 1:] == srt[:, :-1]).any(axis=1)
    if bad.any():
        for bi in np.nonzero(bad)[0]:
            out[bi] = _host_row_reference(query[bi], keys, values.reshape(N, H, F),
                                          decay_full, top_k)
    return out.astype(np.float32)


# revision 21
# speedup vs baseline: 1.5192x; 1.5192x over previous
"""Distributed kNN-retrieval kernel for Trainium2 (8 NeuronCores).

Problem: nn_CHRC_47562467836574 (retrieval_knn).
  corrected[b] = softmax-weighted sum of values rows at the top-16
  decayed cosine similarities between query b and a 100k-entry memory bank.

Strategy (8-way SPMD, bass/Tile):
  * Decay cutoff: timestamps are sorted, and with L2-normalized vectors
    |cos| <= 1, so an entry's decayed sim is bounded by its decay factor
    0.995^age.  Entries older than the cutoff (decay < CUT) can never reach
    a query's top-16 (16th-best sims measure ~0.08 on this distribution).
    The host keeps only the newest slice (rounded up to a full multiple of
    8*TILE_N), trimming ~80% of the matmul work.  A post-hoc host check
    verifies 16th-best >= CUT for every query (exact CPU recompute of any
    violating row — never triggers in practice).
  * Keys: newest slice, sharded contiguously across 8 cores; each core
    computes exact fp32 sims for all 1024 queries x its shard, with
    normalization + decay folded into a per-column key prescale.
  * Local top-16 per query via per-tile vector-engine max8/max_index
    (top-16 per 500-wide tile can never miss a local-top-16 member);
    in-tile positions recovered with a fp32 one-hot gather.
  * AllGather of the 8x(local top-16 sims + global indices); each core
    then reduces its OWN 128-query slice (selected with host-provided
    gather offsets) to the global top-16, softmax-weights it, gathers the
    16 full-width value rows, and writes its [128, 672] output slice.
"""

import math
import os

import numpy as np

CUT = 0.01          # decay cutoff; 16th-best sims ~0.08 on this data (7x margin)
DECAY_FACTOR = 0.995
TEMPERATURE = 0.1
MIN_SIMILARITY = 0.0
EPS = 1e-8

_cache = {}


# ---------------------------------------------------------------------------
# device program
# ---------------------------------------------------------------------------

def build(b, n_loc, n_rows, hf, tile_n=500, n_cores=8, d=512, k=16):
    """Build + compile the SPMD program (same program for every core)."""
    from contextlib import ExitStack

    import concourse.bass as bass
    import concourse.tile as tile
    from concourse import bacc, mybir

    f32 = mybir.dt.float32
    u32 = mybir.dt.uint32
    nt = n_loc // tile_n
    assert n_loc % tile_n == 0
    nb = b // 128
    assert b % 128 == 0
    nt16 = nt * 16
    ln_decay = float(np.log(np.float32(DECAY_FACTOR)))

    nc = bacc.Bacc("TRN2", target_bir_lowering=False, debug=False,
                   num_devices=n_cores)

    qT = nc.dram_tensor("qT", [d, b], f32, kind="ExternalInput")
    kT = nc.dram_tensor("kT", [d, n_loc], f32, kind="ExternalInput")
    age = nc.dram_tensor("age", [1, n_loc], f32, kind="ExternalInput")
    vals = nc.dram_tensor("vals", [n_rows, hf], f32, kind="ExternalInput")
    crow = nc.dram_tensor("crow", [128, 1], u32, kind="ExternalInput")
    iota_t = nc.dram_tensor("iota_t", [1, nt16], f32, kind="ExternalInput")
    # per-core final-stage gather offsets:
    #   grow[:, r] = r*b + core*128 + i      (rows of the AG outputs)
    #   ioc16[i]   = (core*128 + i) * 16     (flat-element base of own row)
    grow = nc.dram_tensor("grow", [128, n_cores], u32, kind="ExternalInput")
    ioc16 = nc.dram_tensor("ioc16", [128, 1], u32, kind="ExternalInput")
    out = nc.dram_tensor("out", [128, hf], f32, kind="ExternalOutput")
    dbg_s = nc.dram_tensor("dbg_s", [128, k], f32, kind="ExternalOutput")
    dbg_i = nc.dram_tensor("dbg_i", [128, k], u32, kind="ExternalOutput")

    dch = d // 128  # contraction chunks

    with tile.TileContext(nc) as tc, ExitStack() as ctx:
        sb = ctx.enter_context(tc.tile_pool(name="sb", bufs=1))
        sb3 = ctx.enter_context(tc.tile_pool(name="sb3", bufs=3))
        sb2 = ctx.enter_context(tc.tile_pool(name="sb2", bufs=2))
        ps = ctx.enter_context(tc.tile_pool(name="ps", bufs=3, space="PSUM"))
        psn = ctx.enter_context(tc.tile_pool(name="psn", bufs=2, space="PSUM"))
        dram = ctx.enter_context(tc.tile_pool(name="dram", bufs=1, space="DRAM"))

        # ---- constants / loads -------------------------------------------
        ones = sb.tile([128, 128], f32, tag="ones")
        nc.vector.memset(ones[:], 1.0)
        crow_s = sb.tile([128, 1], u32, tag="crow")
        nc.sync.dma_start(out=crow_s[:], in_=crow.ap())
        iota_t_s = sb.tile([128, nt16], f32, tag="iota_t")
        nc.sync.dma_start(out=iota_t_s[:], in_=iota_t.ap().to_broadcast([128, nt16]))
        grow_s = sb.tile([128, n_cores], u32, tag="grow")
        nc.sync.dma_start(out=grow_s[:], in_=grow.ap())
        ioc16_s = sb.tile([128, 1], u32, tag="ioc16")
        nc.sync.dma_start(out=ioc16_s[:], in_=ioc16.ap())

        qTs = sb.tile([128, dch, b], f32, tag="qT")
        nc.sync.dma_start(out=qTs[:], in_=qT.ap().rearrange("(c p) b -> p c b", p=128))
        kts = []
        for t in range(nt):
            kt_t = sb.tile([128, dch, tile_n], f32, tag=f"kt{t}")
            nc.sync.dma_start(
                out=kt_t[:],
                in_=kT.ap().rearrange("(c p) n -> p c n", p=128)[
                    :, :, t * tile_n:(t + 1) * tile_n],
            )
            kts.append(kt_t)

        # ---- query normalization -----------------------------------------
        # ones[128,128] stationary => norm sums replicated on all partitions
        qnrm = sb.tile([128, b], f32, tag="qnrm")
        nbt = math.ceil(b / 512)
        for i in range(nbt):
            w = min(512, b - i * 512)
            sq_q = sb2.tile([128, dch, w], f32, tag="sqx", name="sq_q")
            nc.scalar.square(sq_q[:], qTs[:, :, i * 512:i * 512 + w])
            pq = psn.tile([128, w], f32, tag="pn")
            for c in range(dch):
                nc.tensor.matmul(pq[:], ones[:], sq_q[:, c, :],
                                 start=(c == 0), stop=(c == dch - 1))
            nc.scalar.sqrt(qnrm[:, i * 512:i * 512 + w], pq[:])
        nc.vector.tensor_scalar_max(qnrm[:], qnrm[:], 1e-12)
        qinv = sb.tile([128, b], f32, tag="qinv")
        nc.vector.reciprocal(qinv[:], qnrm[:])
        nc.vector.tensor_tensor(
            out=qTs[:], in0=qTs[:],
            in1=qinv[:].unsqueeze(1).to_broadcast([128, dch, b]),
            op=mybir.AluOpType.mult)

        # ---- per-tile key prescale: 1/norm * decay -----------------------
        for t in range(nt):
            kt_t = kts[t]
            sq_k = sb2.tile([128, dch, tile_n], f32, tag="sqx", name="sq_k")
            nc.scalar.square(sq_k[:], kt_t[:])
            pn = psn.tile([128, tile_n], f32, tag="pn")
            for c in range(dch):
                nc.tensor.matmul(pn[:], ones[:], sq_k[:, c, :],
                                 start=(c == 0), stop=(c == dch - 1))
            knrm = sb2.tile([128, tile_n], f32, tag="knrm")
            nc.scalar.sqrt(knrm[:], pn[:])
            nc.vector.tensor_scalar_max(knrm[:], knrm[:], 1e-12)
            kinv = sb2.tile([128, tile_n], f32, tag="kinv")
            nc.vector.reciprocal(kinv[:], knrm[:])
            aget = sb2.tile([128, tile_n], f32, tag="aget")
            nc.sync.dma_start(
                out=aget[:],
                in_=age.ap()[:, t * tile_n:(t + 1) * tile_n]
                    .to_broadcast([128, tile_n]))
            dec = sb2.tile([128, tile_n], f32, tag="dec")
            nc.scalar.activation(dec[:], aget[:],
                                 mybir.ActivationFunctionType.Exp,
                                 bias=0.0, scale=ln_decay)
            nc.vector.tensor_tensor(out=kinv[:], in0=kinv[:], in1=dec[:],
                                    op=mybir.AluOpType.mult)
            nc.vector.tensor_tensor(
                out=kt_t[:], in0=kt_t[:],
                in1=kinv[:].unsqueeze(1).to_broadcast([128, dch, tile_n]),
                op=mybir.AluOpType.mult)

        # ---- sims + local scan -------------------------------------------
        ag_in_s = dram.tile([b, k], f32, tag="ag_in_s")
        ag_in_i = dram.tile([b, k], u32, tag="ag_in_i")

        for bc in range(nb):
            vb = sb3.tile([128, nt16], f32, tag="vb")
            pb = sb3.tile([128, nt16], u32, tag="pb")
            for t in range(nt):
                kt_t = kts[t]
                pt = ps.tile([128, tile_n], f32, tag="p")
                for c in range(dch):
                    nc.tensor.matmul(pt[:], qTs[:, c, bc * 128:(bc + 1) * 128],
                                     kt_t[:, c, :],
                                     start=(c == 0), stop=(c == dch - 1))
                sims = sb3.tile([128, tile_n], f32, tag="sims")
                nc.scalar.copy(sims[:], pt[:])
                nc.vector.max(vb[:, t * 16:t * 16 + 8], sims[:])
                nc.vector.max_index(pb[:, t * 16:t * 16 + 8],
                                    vb[:, t * 16:t * 16 + 8], sims[:])
                scr = sb3.tile([128, tile_n], f32, tag="scr")
                nc.vector.match_replace(scr[:], vb[:, t * 16:t * 16 + 8],
                                        sims[:], -3.0e38)
                nc.vector.max(vb[:, t * 16 + 8:t * 16 + 16], scr[:])
                nc.vector.max_index(pb[:, t * 16 + 8:t * 16 + 16],
                                    vb[:, t * 16 + 8:t * 16 + 16], scr[:])

            # local top-16 of the nt16 candidates
            lv = sb3.tile([128, k], f32, tag="lv")
            nc.vector.max(lv[:, 0:8], vb[:])
            vscr = sb3.tile([128, nt16], f32, tag="vscr")
            nc.vector.match_replace(vscr[:], lv[:, 0:8], vb[:], -3.0e38)
            nc.vector.max(lv[:, 8:16], vscr[:])
            vp = sb3.tile([128, k], u32, tag="vp")
            nc.vector.max_index(vp[:, 0:8], lv[:, 0:8], vb[:])
            nc.vector.max_index(vp[:, 8:16], lv[:, 8:16], vscr[:])
            # one-hot gather lpos[p,j] = pb[p, vp[p,j]] (fp32: integer-exact)
            vp_f = sb3.tile([128, k], f32, tag="vp_f")
            nc.vector.tensor_copy(out=vp_f[:], in_=vp[:])
            pb_f = sb3.tile([128, nt16], f32, tag="pb_f")
            nc.vector.tensor_copy(out=pb_f[:], in_=pb[:])
            cmp = sb2.tile([128, k, nt16], f32, tag="cmp")
            nc.vector.tensor_tensor(
                out=cmp[:],
                in0=vp_f[:].unsqueeze(2).to_broadcast([128, k, nt16]),
                in1=iota_t_s[:].unsqueeze(1).to_broadcast([128, k, nt16]),
                op=mybir.AluOpType.is_equal)
            nc.vector.tensor_tensor(
                out=cmp[:], in0=cmp[:],
                in1=pb_f[:].unsqueeze(1).to_broadcast([128, k, nt16]),
                op=mybir.AluOpType.mult)
            lpos_f = sb3.tile([128, k], f32, tag="lpos_f")
            nc.vector.tensor_reduce(lpos_f[:], cmp[:], axis=mybir.AxisListType.X,
                                    op=mybir.AluOpType.add)
            lpos = sb3.tile([128, k], u32, tag="lpos")
            nc.vector.tensor_copy(out=lpos[:], in_=lpos_f[:])
            # gidx = (vp>>4)*tile_n + lpos + crow
            gidx = sb3.tile([128, k], u32, tag="gidx")
            nc.vector.tensor_scalar(out=gidx[:], in0=vp[:], scalar1=4,
                                    scalar2=None,
                                    op0=mybir.AluOpType.logical_shift_right)
            nc.vector.tensor_scalar(out=gidx[:], in0=gidx[:], scalar1=tile_n,
                                    scalar2=None, op0=mybir.AluOpType.mult)
            nc.vector.tensor_tensor(out=gidx[:], in0=gidx[:], in1=lpos[:],
                                    op=mybir.AluOpType.add)
            nc.vector.tensor_tensor(out=gidx[:], in0=gidx[:],
                                    in1=crow_s[:].to_broadcast([128, k]),
                                    op=mybir.AluOpType.add)
            nc.vector.tensor_scalar_min(gidx[:], gidx[:], n_rows - 1)
            nc.sync.dma_start(out=ag_in_s[bc * 128:(bc + 1) * 128, :], in_=lv[:])
            nc.sync.dma_start(out=ag_in_i[bc * 128:(bc + 1) * 128, :], in_=gidx[:])

        # ---- AllGather ----------------------------------------------------
        ag_out_s = dram.tile([n_cores * b, k], f32, tag="ag_out_s")
        ag_out_i = dram.tile([n_cores * b, k], u32, tag="ag_out_i")
        rg = [list(range(n_cores))]
        nc.gpsimd.collective_compute("AllGather", mybir.AluOpType.bypass,
                                     replica_groups=rg,
                                     ins=[ag_in_s[:].opt()],
                                     outs=[ag_out_s[:].opt()])
        nc.gpsimd.collective_compute("AllGather", mybir.AluOpType.bypass,
                                     replica_groups=rg,
                                     ins=[ag_in_i[:].opt()],
                                     outs=[ag_out_i[:].opt()])

        # ---- final reduction: own 128-query slice only -------------------
        # G[i, r, :] = ag_out_s[r*b + core*128 + i, :]  via input offsets
        G = sb.tile([128, n_cores, k], f32, tag="G")
        for r in range(n_cores):
            nc.gpsimd.indirect_dma_start(
                out=G[:, r, :], out_offset=None,
                in_=ag_out_s[:],
                in_offset=bass.IndirectOffsetOnAxis(ap=grow_s[:, r:r + 1], axis=0))
        Gv = G[:].rearrange("p r k -> p (r k)")
        nck = n_cores * k
        fv = sb.tile([128, k], f32, tag="fv")
        nc.vector.max(fv[:, 0:8], Gv)
        Gscr = sb.tile([128, nck], f32, tag="Gscr")
        nc.vector.match_replace(Gscr[:], fv[:, 0:8], Gv, -3.0e38)
        nc.vector.max(fv[:, 8:16], Gscr[:])
        fp = sb.tile([128, k], u32, tag="fp")
        nc.vector.max_index(fp[:, 0:8], fv[:, 0:8], Gv)
        nc.vector.max_index(fp[:, 8:16], fv[:, 8:16], Gscr[:])
        # flat-element offsets into ag_out_i: (fp>>4)*(b*16) + ioc16 + (fp&15)
        offi = sb.tile([128, k], u32, tag="offi")
        nc.vector.tensor_scalar(out=offi[:], in0=fp[:], scalar1=4, scalar2=None,
                                op0=mybir.AluOpType.logical_shift_right)
        nc.vector.tensor_scalar(out=offi[:], in0=offi[:], scalar1=b * 16,
                                scalar2=None, op0=mybir.AluOpType.mult)
        kk = sb.tile([128, k], u32, tag="kk")
        nc.vector.tensor_scalar(out=kk[:], in0=fp[:], scalar1=15, scalar2=None,
                                op0=mybir.AluOpType.bitwise_and)
        nc.vector.tensor_tensor(out=offi[:], in0=offi[:], in1=kk[:],
                                op=mybir.AluOpType.add)
        nc.vector.tensor_tensor(out=offi[:], in0=offi[:],
                                in1=ioc16_s[:].to_broadcast([128, k]),
                                op=mybir.AluOpType.add)
        fgi = sb.tile([128, k], u32, tag="fgi")
        for j in range(k):
            nc.gpsimd.indirect_dma_start(
                out=fgi[:, j:j + 1], out_offset=None,
                in_=ag_out_i[:].rearrange("q c -> (q c)").unsqueeze(1),
                in_offset=bass.IndirectOffsetOnAxis(ap=offi[:, j:j + 1], axis=0))
        # gather the 16 full-width value rows
        V = sb.tile([128, k, hf], f32, tag="V")
        for j in range(k):
            nc.gpsimd.indirect_dma_start(
                out=V[:, j, :], out_offset=None,
                in_=vals.ap(),
                in_offset=bass.IndirectOffsetOnAxis(ap=fgi[:, j:j + 1], axis=0))
        # softmax weights with MIN_SIMILARITY mask + renorm (ref formula)
        negm = sb.tile([128, 1], f32, tag="negm")
        nc.vector.tensor_scalar_mul(negm[:], fv[:, 0:1], -1.0 / TEMPERATURE)
        e = sb.tile([128, k], f32, tag="e")
        nc.scalar.activation(e[:], fv[:], mybir.ActivationFunctionType.Exp,
                             bias=negm[:], scale=1.0 / TEMPERATURE)
        m = sb.tile([128, k], f32, tag="m")
        nc.vector.tensor_scalar(out=m[:], in0=fv[:], scalar1=MIN_SIMILARITY,
                                scalar2=None, op0=mybir.AluOpType.is_ge)
        em = sb.tile([128, k], f32, tag="em")
        nc.vector.tensor_tensor(out=em[:], in0=e[:], in1=m[:],
                                op=mybir.AluOpType.mult)
        S = sb.tile([128, 1], f32, tag="S")
        nc.vector.tensor_reduce(S[:], e[:], axis=mybir.AxisListType.X,
                                op=mybir.AluOpType.add)
        Sm = sb.tile([128, 1], f32, tag="Sm")
        nc.vector.tensor_reduce(Sm[:], em[:], axis=mybir.AxisListType.X,
                                op=mybir.AluOpType.add)
        den = sb.tile([128, 1], f32, tag="den")
        nc.vector.tensor_scalar(out=den[:], in0=S[:], scalar1=EPS,
                                scalar2=Sm[:], op0=mybir.AluOpType.mult,
                                op1=mybir.AluOpType.add)
        winv = sb.tile([128, 1], f32, tag="winv")
        nc.vector.reciprocal(winv[:], den[:])
        w = sb.tile([128, k], f32, tag="w")
        nc.vector.tensor_scalar(out=w[:], in0=em[:], scalar1=winv[:],
                                scalar2=None, op0=mybir.AluOpType.mult)
        # weighted sum: per-k scale on ACT (in place), reduce on DVE
        for j in range(k):
            nc.scalar.activation(V[:, j, :], V[:, j, :],
                                 mybir.ActivationFunctionType.Copy,
                                 bias=0.0, scale=w[:, j:j + 1])
        acc = sb.tile([128, hf], f32, tag="acc")
        nc.vector.tensor_reduce(acc[:], V[:].transpose([0, 2, 1]),
                                axis=mybir.AxisListType.X, op=mybir.AluOpType.add)
        nc.sync.dma_start(out=out.ap(), in_=acc[:])
        nc.sync.dma_start(out=dbg_s.ap(), in_=fv[:])
        nc.sync.dma_start(out=dbg_i.ap(), in_=fgi[:])

    nc.compile()
    return nc


# ---------------------------------------------------------------------------
# host wrapper
# ---------------------------------------------------------------------------

def _host_row_reference(qrow, keys, values, decay, top_k):
    """Exact CPU recompute of one query row (fallback safety net)."""
    qn = qrow / max(np.linalg.norm(qrow), 1e-12)
    kn = keys / np.maximum(
        np.linalg.norm(keys, axis=1, keepdims=True), 1e-12)
    sims = (kn @ qn).astype(np.float32) * decay
    idx = np.argpartition(-sims, top_k)[:top_k]
    idx = idx[np.argsort(-sims[idx], kind="stable")]
    ts_ = sims[idx]
    e = np.exp((ts_ - ts_.max()) / np.float32(TEMPERATURE))
    sm = e / e.sum()
    msk = ts_ >= MIN_SIMILARITY
    wgt = sm * msk
    wgt = wgt / (wgt.sum() + EPS)
    return np.einsum("k,khf->hf", wgt, values[idx]).astype(np.float32)


def kernel(query, keys, values, timestamps, global_step, top_k):
    from concourse import bass_utils

    query = np.asarray(query, dtype=np.float32)
    keys = np.asarray(keys, dtype=np.float32)
    values = np.asarray(values, dtype=np.float32)
    timestamps = np.asarray(timestamps)
    gs = int(global_step)
    top_k = int(top_k)
    assert top_k == 16, f"kernel compiled for top_k=16, got {top_k}"

    B, D = query.shape
    N = keys.shape[0]
    H, F = values.shape[1], values.shape[2]
    n_cores = 8
    tile_n = 500
    assert B == n_cores * 128 and D == 512
    hf = H * F

    # ---- decay cutoff (sorted timestamps) ----------------------------------
    age_cut = int(math.floor(math.log(CUT) / math.log(DECAY_FACTOR)))
    idx0 = int(np.searchsorted(timestamps, gs - age_cut, side="left"))
    keep = N - idx0
    n_loc = max(tile_n, math.ceil(keep / (n_cores * tile_n)) * tile_n)
    S = N - n_cores * n_loc
    if S < 0:
        S = 0
        n_loc = math.ceil(N / (n_cores * tile_n)) * tile_n

    key = (B, n_loc, N, hf)
    if key not in _cache:
        _cache[key] = build(B, n_loc, N, hf, tile_n=tile_n, n_cores=n_cores)
    nc = _cache[key]

    # ---- host-side input prep ---------------------------------------------
    qT = np.ascontiguousarray(query.T)
    ages = (gs - timestamps).astype(np.float32)
    vals2d = np.ascontiguousarray(values.reshape(N, hf))
    iarange = np.arange(128, dtype=np.uint32)

    in_maps = []
    for c in range(n_cores):
        lo = S + c * n_loc
        hi = lo + n_loc
        if hi <= N:
            ksl = keys[lo:hi]
            asl = ages[lo:hi]
        else:  # padding path (never hit at full scale)
            ksl = np.concatenate([keys[lo:N], np.ones((hi - N, D), np.float32)])
            asl = np.concatenate([ages[lo:N], np.full(hi - N, 1e9, np.float32)])
        in_maps.append({
            "qT": qT,
            "kT": np.ascontiguousarray(ksl.T),
            "age": np.ascontiguousarray(asl[None, :]),
            "vals": vals2d,
            "crow": np.full((128, 1), lo, np.uint32),
            "iota_t": np.arange(n_loc // tile_n * 16, dtype=np.float32)[None, :],
            "grow": np.ascontiguousarray(
                np.arange(n_cores, dtype=np.uint32)[None, :] * B
                + c * 128 + iarange[:, None]),
            "ioc16": ((c * 128 + iarange) * 16).reshape(128, 1).astype(np.uint32),
        })

    trace = os.environ.get("KNN_TRACE", "") == "1"
    res = bass_utils.run_bass_kernel_spmd(
        nc, in_maps, core_ids=list(range(n_cores)), trace=trace)
    kernel.last_exec_time_ns = res.exec_time_ns

    out = np.concatenate([res.results[c]["out"] for c in range(n_cores)],
                         axis=0).reshape(B, H, F)

    # ---- host safety net ---------------------------------------------------
    fv = np.concatenate([res.results[c]["dbg_s"] for c in range(n_cores)])
    fgi = np.concatenate([res.results[c]["dbg_i"] for c in range(n_cores)])
    decay_full = np.power(np.float32(DECAY_FACTOR), ages).astype(np.float32)
    bad = (fv[:, top_k - 1] < CUT)
    srt = np.sort(fgi, axis=1)
    bad |= (srt[:, 1:] == srt[:, :-1]).any(axis=1)
    if bad.any():
        vals3d = values.reshape(N, H, F)
        for bi in np.nonzero(bad)[0]:
            out[bi] = _host_row_reference(query[bi], keys, vals3d,
                                          decay_full, top_k)
    return out.astype(np.float32)
